# revision 20
# baseline (speedup 1.0000x reference)
"""Trainium2 Bass kernel for nn_ContractExpand (segment_reduce, 5 scales).

out[n, b, l, e] = relu(segsum_r(x)[b, g(l), :] @ (W[n]/r).T + b[n]/r) broadcast
over groups.  Data-parallel over B across 8 cores (8 batches each).

v3 design (uniform 128-contraction matmuls at full PE clock):
 - host: transpose x to xt[k, p, b, l] (three zero-PADDED 128-row d-slices;
   row d=300 is the ones column folding the bias: wt_aug[n] = [W[n].T/r ;
   b[n]/r^2 ; 0-pad]).  Sub-128 contraction locks the PE at 1.2GHz and mixed
   PE tile sizes add ~115ns/switch (measured), so every matmul is exactly
   [K=128, M=128, N=300] bf16 -> 125ns streaming at 2.4GHz.
 - device:
     * r=1 stationary windows slice xt directly (transpose is free).
     * seg sums: Pool(gpsimd) computes r2 (strided even+odd add from xt) and
       r4 (from seg2); DVE computes r10 (strided reduce from seg2) and r25
       (reduce from xt).  Packed bf16 seg tiles [128, 5696].
     * main matmul: 95 windows x 3 k-tiles into paired PSUM banks (bufs=4).
     * ReLU evac PSUM -> one fp16 y tile [128, 95, 300]; ACT engine mostly,
       DVE takes late pairs after its seg queue drains.
     * compact fp16 stores (13 contiguous chunks, sync ring, big-to-small);
       r-fold row replication + f32 upcast happens on host during unshard.
 - loads: need-ordered batch chunks, k0/k2+wt on sync ring, k1 on gpsimd
   ring (per-ring transfers serialize; a single dma_start runs ~350GB/s).
"""

import numpy as np
import ml_dtypes

import concourse.bass as bass
import concourse.tile as tile
from concourse import bacc, mybir
from concourse.bass_utils import run_bass_kernel_spmd

F32 = mybir.dt.float32
F16 = mybir.dt.float16
BF16 = mybir.dt.bfloat16

R_SCALES = (1, 2, 4, 10, 25)
B, L, D = 64, 800, 300
DP = 384                                              # padded d (3 x 128)
NCORES = 8
B_LOC = B // NCORES                                   # 8 batches per core
G = [L // r for r in R_SCALES]                        # 800 400 200 80 32
G8 = [g * B_LOC for g in G]                           # 6400 3200 1600 640 256
OFF8 = np.cumsum([0] + G8).tolist()                   # out row offsets
GTOT8 = OFF8[-1]                                      # 12096
# seg tile column blocks for scales r>=2 (batch-major inside each block)
SOFF = np.cumsum([0] + G8[1:]).tolist()               # 0 3200 4800 5440 5696
SEGW = SOFF[-1]                                       # 5696

# main-matmul windows: per scale, ceil(G8/128) windows; stationary is ALWAYS
# 128 cols (the r4 tail window reads 64 junk cols whose out rows aren't
# stored), so every MM is uniform [128, 128, 300].
UNITS = []  # (n, col0_within_scale, gw_store)
for n in range(5):
    c = 0
    while c < G8[n]:
        gw = min(128, G8[n] - c)
        UNITS.append((n, c, gw))
        c += gw
NU = len(UNITS)                                       # 95

PAIRS = []  # consecutive same-scale full-width units share a psum pair tile
_i = 0
while _i < NU:
    n, c0, gw = UNITS[_i]
    if _i + 1 < NU and UNITS[_i + 1][0] == n and gw == 128 and UNITS[_i + 1][2] == 128:
        PAIRS.append([_i, _i + 1])
        _i += 2
    else:
        PAIRS.append([_i])
        _i += 1

SCH = 10          # store chunk: units per DMA store
PSUM_BUFS = 4     # pair tiles (2 banks each)
EVAC_ACT_ONLY = 38  # pairs before this index evac on ACT; later alternate DVE


def build_wt_aug(W, b):
    out = np.zeros((5, DP, D), np.float64)
    for n, r in enumerate(R_SCALES):
        out[n, :D, :] = np.asarray(W[n], np.float64).T / r
        out[n, D, :] = np.asarray(b[n], np.float64) / (r * r)
    return out.astype(ml_dtypes.bfloat16)


def _body(tc, out_ap, xt_ap, wt_ap):
    nc = tc.nc
    with (
        tc.tile_pool(name="consts", bufs=1) as consts,
        tc.tile_pool(name="xtp", bufs=1) as xtp,
        tc.tile_pool(name="segp", bufs=1) as segp,
        tc.tile_pool(name="yp", bufs=1) as yp,
        tc.tile_pool(name="psp", bufs=PSUM_BUFS, space="PSUM") as psp,
    ):
        # Loads: DMA completion is ring-ordered, so the chain BEFORE the first
        # matmul must be minimal: only batch-0/1 chunks and the n=0 weight
        # slices are emitted upfront (3 rings in parallel); everything else is
        # emitted lazily inside the main loop, always before its first
        # consumer and before its deadline on the serialized ring.
        wall = [consts.tile([128, 5, D], BF16, name=f"wall_{k}") for k in range(3)]
        xt = [xtp.tile([128, B_LOC, L], BF16, name=f"xt_{k}") for k in range(3)]

        def load_wt(n, ring):
            for k in range(3):
                ring.dma_start(
                    out=wall[k][:, n, :],
                    in_=wt_ap[n, k * 128 : (k + 1) * 128, :],
                )

        def load_xt(k, b0, nb, ring):
            ring.dma_start(
                out=xt[k][:, b0 : b0 + nb, :],
                in_=xt_ap[k, :, b0 : b0 + nb, :],
            )

        load_wt(0, nc.scalar)
        load_xt(2, 0, 1, nc.scalar)
        load_xt(2, 1, 1, nc.scalar)
        load_xt(0, 0, 1, nc.sync)
        load_xt(0, 1, 1, nc.sync)
        load_xt(1, 0, 1, nc.gpsimd)
        load_xt(1, 1, 1, nc.gpsimd)

        def lazy(pi):
            if pi == 2:
                load_xt(0, 2, 2, nc.sync)
                load_xt(2, 2, 2, nc.sync)
                load_xt(1, 2, 2, nc.gpsimd)
            elif pi == 4:
                load_wt(1, nc.sync)
            elif pi == 5:
                load_xt(0, 4, 2, nc.sync)
                load_xt(2, 4, 2, nc.sync)
                load_xt(1, 4, 2, nc.gpsimd)
            elif pi == 8:
                load_xt(0, 6, 2, nc.sync)
                load_xt(2, 6, 2, nc.sync)
                load_xt(1, 6, 2, nc.gpsimd)
            elif pi == 12:
                load_wt(2, nc.sync)
            elif pi == 16:
                load_wt(3, nc.sync)
            elif pi == 18:
                load_wt(4, nc.sync)

        seg = [segp.tile([128, SEGW], BF16, name=f"seg_{k}") for k in range(3)]
        y = yp.tile([128, NU, D], F16, name="y")

        # ---- seg ops, 2 batches per op, emitted interleaved with the main
        # loop.  Pool: r2 (even+odd strided add from xt) then r4 (from seg2).
        # DVE: r10 (reduce from seg2) and r25 (reduce from xt).
        def pool_seg_ops():
            with nc.allow_low_precision(reason="bf16 segment sums (tol 2e-2)"):
                for b0 in range(0, B_LOC, 2):
                    for k in range(3):
                        src = xt[k][:, b0 : b0 + 2, :].rearrange(
                            "p b (g r) -> p b g r", r=2
                        )
                        dst = seg[k][:, b0 * 400 : (b0 + 2) * 400].rearrange(
                            "p (b g) -> p b g", b=2
                        )
                        nc.gpsimd.tensor_add(dst, src[:, :, :, 0], src[:, :, :, 1])
                        yield
                for b0 in range(0, B_LOC, 2):
                    for k in range(3):
                        s2 = seg[k][:, b0 * 400 : (b0 + 2) * 400].rearrange(
                            "p (b g r) -> p b g r", b=2, r=2
                        )
                        dst = seg[k][
                            :, SOFF[1] + b0 * 200 : SOFF[1] + (b0 + 2) * 200
                        ].rearrange("p (b g) -> p b g", b=2)
                        nc.gpsimd.tensor_add(dst, s2[:, :, :, 0], s2[:, :, :, 1])
                        yield

        def dve_seg_ops():
            with nc.allow_low_precision(reason="bf16 segment sums (tol 2e-2)"):
                for b0 in range(0, B_LOC, 2):
                    for k in range(3):
                        # r10 from seg2 (groups of 5 adjacent seg2 cols)
                        nc.vector.tensor_reduce(
                            seg[k][
                                :, SOFF[2] + b0 * 80 : SOFF[2] + (b0 + 2) * 80
                            ].rearrange("p (b g) -> p b g", b=2),
                            seg[k][:, b0 * 400 : (b0 + 2) * 400].rearrange(
                                "p (b g r) -> p b g r", b=2, r=5
                            ),
                            axis=mybir.AxisListType.X,
                            op=mybir.AluOpType.add,
                        )
                        yield
                        # r25 straight from xt
                        nc.vector.tensor_reduce(
                            seg[k][
                                :, SOFF[3] + b0 * 32 : SOFF[3] + (b0 + 2) * 32
                            ].rearrange("p (b g) -> p b g", b=2),
                            xt[k][:, b0 : b0 + 2, :].rearrange(
                                "p b (g r) -> p b g r", r=25
                            ),
                            axis=mybir.AxisListType.X,
                            op=mybir.AluOpType.add,
                        )
                        yield

        pool_it = pool_seg_ops()
        dve_it = dve_seg_ops()

        def stationary(n, k, c0):
            """Always a 128-col window; precise APs so Tile dep-tracking stays
            chunk-granular (no whole-tile rearrange)."""
            if n == 0:
                b0, b1 = c0 // L, (c0 + 127) // L
                if b0 == b1:
                    return xt[k][:, b0, c0 - b0 * L : c0 - b0 * L + 128]
                return xt[k][:, b0 : b0 + 2, :].rearrange("p b l -> p (b l)")[
                    :, c0 - b0 * L : c0 - b0 * L + 128
                ]
            return seg[k][:, SOFF[n - 1] + c0 : SOFF[n - 1] + c0 + 128]

        # ---- main loop over psum pairs ----
        # big early chunks store on the sync ring; the last small chunks go
        # to the gpsimd/scalar rings (free by then) to cut the serial tail
        pending_stores = []
        n_stores = 0

        def flush_stores(force=False, ring=None):
            nonlocal pending_stores, n_stores
            ring = ring or nc.sync
            while pending_stores:
                full = [u for (n_, u) in pending_stores if UNITS[u][2] == 128]
                if full and (len(full) >= SCH or force):
                    n0, u0 = pending_stores[0]
                    nj = len(full)
                    r0 = OFF8[n0] + UNITS[u0][1]
                    ring.dma_start(
                        out=out_ap[r0 : r0 + nj * 128].rearrange(
                            "(j p) e -> p j e", p=128
                        ),
                        in_=y[:, u0 : u0 + nj, :],
                    )
                    n_stores += 1
                    pending_stores = pending_stores[nj:]
                    continue
                if pending_stores and UNITS[pending_stores[0][1]][2] != 128:
                    n_, u_ = pending_stores[0]
                    gw = UNITS[u_][2]
                    r0 = OFF8[n_] + UNITS[u_][1]
                    ring.dma_start(
                        out=out_ap[r0 : r0 + gw], in_=y[0:gw, u_, :]
                    )
                    n_stores += 1
                    pending_stores = pending_stores[1:]
                    continue
                break

        ui = 0
        for pi, pair in enumerate(PAIRS):
            lazy(pi)
            # interleave seg-op emission: one per engine per pair until done
            next(pool_it, None)
            next(dve_it, None)
            ps = psp.tile([128, 1024], F32, name="mainps", tag="mainps")
            for j, u in enumerate(pair):
                n, c0, gw = UNITS[u]
                for k in range(3):
                    nc.tensor.matmul(
                        ps[0:128, j * 512 : j * 512 + D],
                        stationary(n, k, c0),
                        wall[k][:, n, :],
                        start=(k == 0),
                        stop=(k == 2),
                    )
            nj = len(pair)
            u0 = pair[0]
            gw_min = min(UNITS[u][2] for u in pair)
            src = ps[0:gw_min, :].rearrange("p (j c) -> p j c", c=512)[:, 0:nj, 0:D]
            dst = y[0:gw_min, u0 : u0 + nj, :]
            if pi < EVAC_ACT_ONLY or pi % 2 == 1:
                nc.scalar.activation(dst, src, mybir.ActivationFunctionType.Relu)
            else:
                nc.vector.tensor_scalar_max(dst, src, 0.0)
            for u in pair:
                pending_stores.append((UNITS[u][0], u))
            ui += nj
            scale_end = (ui == NU) or (UNITS[ui][0] != UNITS[ui - 1][0])
            if len(pending_stores) >= SCH or scale_end:
                ring = nc.sync
                if ui > NU - 8:      # r10 tail chunk and later
                    ring = nc.gpsimd
                if ui == NU:         # the final r25 chunk
                    ring = nc.scalar
                flush_stores(force=scale_end, ring=ring)
        for _ in pool_it:
            pass
        for _ in dve_it:
            pass
        flush_stores(force=True, ring=nc.scalar)
        assert not pending_stores


def build_module():
    nc = bacc.Bacc("TRN2", target_bir_lowering=False, debug=False)
    xt = nc.dram_tensor("xt", [3, 128, B_LOC, L], BF16, kind="ExternalInput")
    wt = nc.dram_tensor("wt", [5, DP, D], BF16, kind="ExternalInput")
    out = nc.dram_tensor("out", [GTOT8, D], F16, kind="ExternalOutput")
    with tile.TileContext(nc) as tc:
        _body(tc, out.ap(), xt.ap(), wt.ap())
    nc.compile()
    return nc


_MODULE = None


def _get_module():
    global _MODULE
    if _MODULE is None:
        _MODULE = build_module()
    return _MODULE


def make_in_maps(inputs_c_e, W, b):
    x = np.asarray(inputs_c_e, np.float32)
    wt = build_wt_aug(W, b)
    # xt[(k p), b_all, l]; row d=300 is the ones bias column, rest zero-pad
    xt_all = np.zeros((DP, B, L), np.float32)
    xt_all[:D] = x.transpose(2, 0, 1)
    xt_all[D] = 1.0
    xt_all = xt_all.astype(ml_dtypes.bfloat16).reshape(3, 128, B, L)
    return [
        {
            "xt": np.ascontiguousarray(xt_all[:, :, c * B_LOC : (c + 1) * B_LOC]),
            "wt": wt,
        }
        for c in range(NCORES)
    ]


def expand_core_out(o):
    """[12096, 300] f16 compact rows -> [5, B_LOC, L, D] f32."""
    full = np.empty((5, B_LOC, L, D), np.float32)
    for n, r in enumerate(R_SCALES):
        blk = o[OFF8[n] : OFF8[n + 1]].reshape(B_LOC, G[n], D)
        full[n].reshape(B_LOC, G[n], r, D)[...] = blk[:, :, None, :]
    return full


def kernel(inputs_c_e, W, b):
    nc = _get_module()
    in_maps = make_in_maps(inputs_c_e, W, b)
    res = run_bass_kernel_spmd(nc, in_maps, core_ids=list(range(NCORES)))
    out = np.empty((5, B, L, D), np.float32)
    for c in range(NCORES):
        out[:, c * B_LOC : (c + 1) * B_LOC] = expand_core_out(res.results[c]["out"])
    return out


# revision 21
# speedup vs baseline: 1.1021x; 1.1021x over previous
"""Trainium2 Bass kernel for nn_ContractExpand (segment_reduce, 5 scales).

out[n, b, l, e] = relu(segsum_r(x)[b, g(l), :] @ (W[n]/r).T + b[n]/r) broadcast
over groups.  Data-parallel over B across 8 cores (8 batches each).

v3 design (uniform 128-contraction matmuls at full PE clock):
 - host: transpose x to xt[k, p, b, l] (three zero-PADDED 128-row d-slices;
   row d=300 is the ones column folding the bias: wt_aug[n] = [W[n].T/r ;
   b[n]/r^2 ; 0-pad]).  Sub-128 contraction locks the PE at 1.2GHz and mixed
   PE tile sizes add ~115ns/switch (measured), so every matmul is exactly
   [K=128, M=128, N=300] bf16 -> 125ns streaming at 2.4GHz.
 - device:
     * r=1 stationary windows slice xt directly (transpose is free).
     * seg sums: Pool(gpsimd) computes r2 (strided even+odd add from xt) and
       r4 (from seg2); DVE computes r10 (strided reduce from seg2) and r25
       (reduce from xt).  Packed bf16 seg tiles [128, 5696].
     * main matmul: 95 windows x 3 k-tiles into paired PSUM banks (bufs=4).
     * ReLU evac PSUM -> one fp16 y tile [128, 95, 300]; ACT engine mostly,
       DVE takes late pairs after its seg queue drains.
     * compact fp16 stores (13 contiguous chunks, sync ring, big-to-small);
       r-fold row replication + f32 upcast happens on host during unshard.
 - loads: need-ordered batch chunks, k0/k2+wt on sync ring, k1 on gpsimd
   ring (per-ring transfers serialize; a single dma_start runs ~350GB/s).
"""

import numpy as np
import ml_dtypes

import concourse.bass as bass
import concourse.tile as tile
from concourse import bacc, mybir
from concourse.bass_utils import run_bass_kernel_spmd

F32 = mybir.dt.float32
F16 = mybir.dt.float16
BF16 = mybir.dt.bfloat16

R_SCALES = (1, 2, 4, 10, 25)
B, L, D = 64, 800, 300
DP = 384                                              # padded d (3 x 128)
NCORES = 8
B_LOC = B // NCORES                                   # 8 batches per core
G = [L // r for r in R_SCALES]                        # 800 400 200 80 32
G8 = [g * B_LOC for g in G]                           # 6400 3200 1600 640 256
OFF8 = np.cumsum([0] + G8).tolist()                   # out row offsets
GTOT8 = OFF8[-1]                                      # 12096
# seg tile column blocks for scales r>=2 (batch-major inside each block)
SOFF = np.cumsum([0] + G8[1:]).tolist()               # 0 3200 4800 5440 5696
SEGW = SOFF[-1]                                       # 5696

# main-matmul windows: per scale, ceil(G8/128) windows; stationary is ALWAYS
# 128 cols (the r4 tail window reads 64 junk cols whose out rows aren't
# stored), so every MM is uniform [128, 128, 300].
UNITS = []  # (n, col0_within_scale, gw_store)
for n in range(5):
    c = 0
    while c < G8[n]:
        gw = min(128, G8[n] - c)
        UNITS.append((n, c, gw))
        c += gw
NU = len(UNITS)                                       # 95

PAIRS = []  # consecutive same-scale full-width units share a psum pair tile
_i = 0
while _i < NU:
    n, c0, gw = UNITS[_i]
    if _i + 1 < NU and UNITS[_i + 1][0] == n and gw == 128 and UNITS[_i + 1][2] == 128:
        PAIRS.append([_i, _i + 1])
        _i += 2
    else:
        PAIRS.append([_i])
        _i += 1

SCH = 10          # store chunk: units per DMA store
PSUM_BUFS = 4     # pair tiles (2 banks each)
EVAC_ACT_ONLY = 38  # pairs before this index evac on ACT; later alternate DVE


def build_wt_aug(W, b):
    out = np.zeros((5, DP, D), np.float64)
    for n, r in enumerate(R_SCALES):
        out[n, :D, :] = np.asarray(W[n], np.float64).T / r
        out[n, D, :] = np.asarray(b[n], np.float64) / (r * r)
    return out.astype(ml_dtypes.bfloat16)


def _body(tc, out_ap, xt_ap, wt_ap):
    nc = tc.nc
    with (
        tc.tile_pool(name="consts", bufs=1) as consts,
        tc.tile_pool(name="xtp", bufs=1) as xtp,
        tc.tile_pool(name="segp", bufs=1) as segp,
        tc.tile_pool(name="yp", bufs=1) as yp,
        tc.tile_pool(name="psp", bufs=PSUM_BUFS, space="PSUM") as psp,
    ):
        # Loads: DMA completion is ring-ordered, so the chain BEFORE the first
        # matmul must be minimal: only batch-0/1 chunks and the n=0 weight
        # slices are emitted upfront (3 rings in parallel); everything else is
        # emitted lazily inside the main loop, always before its first
        # consumer and before its deadline on the serialized ring.
        wall = [consts.tile([128, 5, D], BF16, name=f"wall_{k}") for k in range(3)]
        xt = [xtp.tile([128, B_LOC, L], BF16, name=f"xt_{k}") for k in range(3)]

        def load_wt(n, ring):
            for k in range(3):
                ring.dma_start(
                    out=wall[k][:, n, :],
                    in_=wt_ap[n, k * 128 : (k + 1) * 128, :],
                )

        def load_xt(k, b0, nb, ring):
            ring.dma_start(
                out=xt[k][:, b0 : b0 + nb, :],
                in_=xt_ap[k, :, b0 : b0 + nb, :],
            )

        load_wt(0, nc.scalar)
        load_xt(2, 0, 1, nc.scalar)
        load_xt(2, 1, 1, nc.scalar)
        load_xt(0, 0, 1, nc.sync)
        load_xt(0, 1, 1, nc.sync)
        load_xt(1, 0, 1, nc.gpsimd)
        load_xt(1, 1, 1, nc.gpsimd)

        # rings are FIFO per transfer; gpsimd's queue is blocked by Pool seg
        # compute, so later k1 chunks ride the sync ring and k2 chunks the
        # scalar ring (issued between the early, un-backlogged evacs)
        def lazy(pi):
            if pi == 2:
                load_xt(0, 2, 2, nc.sync)
                load_xt(1, 2, 2, nc.sync)
                load_xt(2, 2, 2, nc.scalar)
            elif pi == 4:
                load_xt(2, 4, 2, nc.scalar)
            elif pi == 5:
                load_xt(0, 4, 2, nc.sync)
                load_xt(1, 4, 2, nc.sync)
            elif pi == 7:
                load_xt(0, 6, 2, nc.sync)
                load_xt(1, 6, 2, nc.sync)
                load_xt(2, 6, 2, nc.scalar)
            elif pi == 10:
                load_wt(1, nc.sync)
            elif pi == 14:
                load_wt(2, nc.sync)
            elif pi == 16:
                load_wt(3, nc.sync)
                load_wt(4, nc.sync)

        seg = [segp.tile([128, SEGW], BF16, name=f"seg_{k}") for k in range(3)]
        y = yp.tile([128, NU, D], F16, name="y")

        # ---- seg ops, 2 batches per op, emitted interleaved with the main
        # loop.  Pool: r2 (even+odd strided add from xt) then r4 (from seg2).
        # DVE: r10 (reduce from seg2) and r25 (reduce from xt).
        def pool_seg_ops():
            with nc.allow_low_precision(reason="bf16 segment sums (tol 2e-2)"):
                for b0 in range(0, B_LOC, 2):
                    for k in range(3):
                        src = xt[k][:, b0 : b0 + 2, :].rearrange(
                            "p b (g r) -> p b g r", r=2
                        )
                        dst = seg[k][:, b0 * 400 : (b0 + 2) * 400].rearrange(
                            "p (b g) -> p b g", b=2
                        )
                        nc.gpsimd.tensor_add(dst, src[:, :, :, 0], src[:, :, :, 1])
                        yield
                for b0 in range(0, B_LOC, 2):
                    for k in range(3):
                        s2 = seg[k][:, b0 * 400 : (b0 + 2) * 400].rearrange(
                            "p (b g r) -> p b g r", b=2, r=2
                        )
                        dst = seg[k][
                            :, SOFF[1] + b0 * 200 : SOFF[1] + (b0 + 2) * 200
                        ].rearrange("p (b g) -> p b g", b=2)
                        nc.gpsimd.tensor_add(dst, s2[:, :, :, 0], s2[:, :, :, 1])
                        yield

        def dve_seg_ops():
            with nc.allow_low_precision(reason="bf16 segment sums (tol 2e-2)"):
                for b0 in range(0, B_LOC, 2):
                    for k in range(3):
                        # r10 from seg2 (groups of 5 adjacent seg2 cols)
                        nc.vector.tensor_reduce(
                            seg[k][
                                :, SOFF[2] + b0 * 80 : SOFF[2] + (b0 + 2) * 80
                            ].rearrange("p (b g) -> p b g", b=2),
                            seg[k][:, b0 * 400 : (b0 + 2) * 400].rearrange(
                                "p (b g r) -> p b g r", b=2, r=5
                            ),
                            axis=mybir.AxisListType.X,
                            op=mybir.AluOpType.add,
                        )
                        yield
                        # r25 straight from xt
                        nc.vector.tensor_reduce(
                            seg[k][
                                :, SOFF[3] + b0 * 32 : SOFF[3] + (b0 + 2) * 32
                            ].rearrange("p (b g) -> p b g", b=2),
                            xt[k][:, b0 : b0 + 2, :].rearrange(
                                "p b (g r) -> p b g r", r=25
                            ),
                            axis=mybir.AxisListType.X,
                            op=mybir.AluOpType.add,
                        )
                        yield

        pool_it = pool_seg_ops()
        dve_it = dve_seg_ops()

        def stationary(n, k, c0):
            """Always a 128-col window; precise APs so Tile dep-tracking stays
            chunk-granular (no whole-tile rearrange)."""
            if n == 0:
                b0, b1 = c0 // L, (c0 + 127) // L
                if b0 == b1:
                    return xt[k][:, b0, c0 - b0 * L : c0 - b0 * L + 128]
                return xt[k][:, b0 : b0 + 2, :].rearrange("p b l -> p (b l)")[
                    :, c0 - b0 * L : c0 - b0 * L + 128
                ]
            return seg[k][:, SOFF[n - 1] + c0 : SOFF[n - 1] + c0 + 128]

        # ---- main loop over psum pairs ----
        # big early chunks store on the sync ring; the last small chunks go
        # to the gpsimd/scalar rings (free by then) to cut the serial tail
        pending_stores = []
        n_stores = 0

        def flush_stores(force=False, ring=None):
            nonlocal pending_stores, n_stores
            ring = ring or nc.sync
            while pending_stores:
                full = [u for (n_, u) in pending_stores if UNITS[u][2] == 128]
                if full and (len(full) >= SCH or force):
                    n0, u0 = pending_stores[0]
                    nj = len(full)
                    r0 = OFF8[n0] + UNITS[u0][1]
                    ring.dma_start(
                        out=out_ap[r0 : r0 + nj * 128].rearrange(
                            "(j p) e -> p j e", p=128
                        ),
                        in_=y[:, u0 : u0 + nj, :],
                    )
                    n_stores += 1
                    pending_stores = pending_stores[nj:]
                    continue
                if pending_stores and UNITS[pending_stores[0][1]][2] != 128:
                    n_, u_ = pending_stores[0]
                    gw = UNITS[u_][2]
                    r0 = OFF8[n_] + UNITS[u_][1]
                    ring.dma_start(
                        out=out_ap[r0 : r0 + gw], in_=y[0:gw, u_, :]
                    )
                    n_stores += 1
                    pending_stores = pending_stores[1:]
                    continue
                break

        ui = 0
        for pi, pair in enumerate(PAIRS):
            lazy(pi)
            # interleave seg-op emission: one per engine per pair until done
            next(pool_it, None)
            next(dve_it, None)
            ps = psp.tile([128, 1024], F32, name="mainps", tag="mainps")
            for j, u in enumerate(pair):
                n, c0, gw = UNITS[u]
                for k in range(3):
                    nc.tensor.matmul(
                        ps[0:128, j * 512 : j * 512 + D],
                        stationary(n, k, c0),
                        wall[k][:, n, :],
                        start=(k == 0),
                        stop=(k == 2),
                    )
            nj = len(pair)
            u0 = pair[0]
            gw_min = min(UNITS[u][2] for u in pair)
            src = ps[0:gw_min, :].rearrange("p (j c) -> p j c", c=512)[:, 0:nj, 0:D]
            dst = y[0:gw_min, u0 : u0 + nj, :]
            if pi < EVAC_ACT_ONLY or pi % 2 == 1:
                nc.scalar.activation(dst, src, mybir.ActivationFunctionType.Relu)
            else:
                nc.vector.tensor_scalar_max(dst, src, 0.0)
            for u in pair:
                pending_stores.append((UNITS[u][0], u))
            ui += nj
            scale_end = (ui == NU) or (UNITS[ui][0] != UNITS[ui - 1][0])
            if len(pending_stores) >= SCH or scale_end:
                ring = nc.sync
                if ui > NU - 8:      # r10 tail chunk and later
                    ring = nc.gpsimd
                if ui == NU:         # the final r25 chunk
                    ring = nc.scalar
                flush_stores(force=scale_end, ring=ring)
        for _ in pool_it:
            pass
        for _ in dve_it:
            pass
        flush_stores(force=True, ring=nc.scalar)
        assert not pending_stores


def build_module():
    nc = bacc.Bacc("TRN2", target_bir_lowering=False, debug=False)
    xt = nc.dram_tensor("xt", [3, 128, B_LOC, L], BF16, kind="ExternalInput")
    wt = nc.dram_tensor("wt", [5, DP, D], BF16, kind="ExternalInput")
    out = nc.dram_tensor("out", [GTOT8, D], F16, kind="ExternalOutput")
    with tile.TileContext(nc) as tc:
        _body(tc, out.ap(), xt.ap(), wt.ap())
    nc.compile()
    return nc


_MODULE = None


def _get_module():
    global _MODULE
    if _MODULE is None:
        _MODULE = build_module()
    return _MODULE


def make_in_maps(inputs_c_e, W, b):
    x = np.asarray(inputs_c_e, np.float32)
    wt = build_wt_aug(W, b)
    # xt[(k p), b_all, l]; row d=300 is the ones bias column, rest zero-pad
    xt_all = np.zeros((DP, B, L), np.float32)
    xt_all[:D] = x.transpose(2, 0, 1)
    xt_all[D] = 1.0
    xt_all = xt_all.astype(ml_dtypes.bfloat16).reshape(3, 128, B, L)
    return [
        {
            "xt": np.ascontiguousarray(xt_all[:, :, c * B_LOC : (c + 1) * B_LOC]),
            "wt": wt,
        }
        for c in range(NCORES)
    ]


def expand_core_out(o):
    """[12096, 300] f16 compact rows -> [5, B_LOC, L, D] f32."""
    full = np.empty((5, B_LOC, L, D), np.float32)
    for n, r in enumerate(R_SCALES):
        blk = o[OFF8[n] : OFF8[n + 1]].reshape(B_LOC, G[n], D)
        full[n].reshape(B_LOC, G[n], r, D)[...] = blk[:, :, None, :]
    return full


def kernel(inputs_c_e, W, b):
    nc = _get_module()
    in_maps = make_in_maps(inputs_c_e, W, b)
    res = run_bass_kernel_spmd(nc, in_maps, core_ids=list(range(NCORES)))
    out = np.empty((5, B, L, D), np.float32)
    for c in range(NCORES):
        out[:, c * B_LOC : (c + 1) * B_LOC] = expand_core_out(res.results[c]["out"])
    return out
